# revision 14
# baseline (speedup 1.0000x reference)
"""Trainium2 Bass kernel for AdvancedClinicalSafetyLoss.

Strategy: pure data parallel over 8 NeuronCores, with host-side class
bucketing as the sharding layout. The loss is an order-independent sum
over samples, so inside each core's shard the host groups samples by
target class into three fixed-capacity column segments (padded with the
neutral sample x=(0,0,0) whose contribution is replayed exactly on the
host). Every device tile is then single-class at COMPILE TIME:

  - x_t gather disappears (ce = lse - x_cls directly),
  - all target masks disappear,
  - every masked sum (w_t*ce, per-(t,pred) penalty counts, critical
    hits) collapses into per-class plain sums accumulated for free by
    tensor_scalar accum_out.

Device math per tile of class c (logits in bf16, 2x/4x DVE modes):
  e = exp(x)  (ACT, one op over [128, 3F])    s = e0+e1+e2  (gpsimd)
  lse = ln(s) (bf16)                          ce = lse - x_c      [TT]
  pt = exp(-ce) (ACT)                         q = (1-pt)^2  (ACT or DVE)
  fo = q*ce [TT]                              c12 = x1>=x2 [TT]
  m12 = max(x1,x2) [TT]                       c0 = m12 > x0 [TT]
  is1 = c0*c12 [TT]   (pred==1; pred==2 = c0-is1 via host algebra)
Per-tile accums (f32): sum(ce), sum(fo), sum(is1), sum(c0).

Host (float64) reconstructs: weighted-CE ratio from per-class ce sums,
focal mean, penalty matrix bilinear over per-class is1/c0 sums, and the
critical-miss ratio; class counts come from the host's own bucketing.
"""

from contextlib import ExitStack

import numpy as np
import ml_dtypes

import concourse.bass as bass
import concourse.tile as tile
from concourse import bacc, mybir
from concourse import bass_utils

B = 8388608
NCORES = 8
P = 128
BC = B // NCORES            # samples per core
FT = 1408                   # columns per tile
TPC = 2                     # tiles per class segment
CAPC = FT * TPC             # columns per class segment (2816)
NTILES = 3 * TPC
NACC = 4                    # ce, fo, is1, c0 per tile

ALPHA = 0.25
CRIT_PENALTY = 50.0

BF16 = ml_dtypes.bfloat16

_nc_cache = {}


def _patch_act_tables():
    """Make exp/ln/square resolve to the one table set that holds all
    three (natural_log_exp_and_others) so the ACT engine does a single
    table load instead of thrashing between per-function sets.

    The set-id is positional (index into act_info.json), so the dict
    ORDER must be preserved; we instead strip these functions from every
    other set so the placement pass can only pick the combined one."""
    import concourse.bacc as bacc_mod
    import concourse.hw_specs as hw_specs
    if getattr(bacc_mod.get_activation_tables, "_combined_only", False):
        return
    orig = hw_specs.get_activation_tables
    AF = mybir.ActivationFunctionType
    moved = {AF.Exp, AF.Ln, AF.Square}
    pref = "natural_log_exp_and_others"

    def stripped(arch):
        t = orig(arch)
        if pref not in t or not moved <= t[pref]:
            return t
        return {k: (v if k == pref else v - moved) for k, v in t.items()}

    stripped._combined_only = True
    bacc_mod.get_activation_tables = stripped


def _build(repeat: int = 1, timing_loop: bool = False):
    """Build + compile the per-core Bass program (SPMD, same on all cores)."""
    _patch_act_tables()
    f32 = mybir.dt.float32
    bf16 = mybir.dt.bfloat16
    A = mybir.AluOpType
    AF = mybir.ActivationFunctionType

    nc = bacc.Bacc("TRN2", target_bir_lowering=False, debug=False,
                   num_devices=NCORES)
    # xt_d row layout per partition: [NTILES, 3 logit-streams, FT] so each
    # tile is one contiguous [P, 3*FT] DMA.
    xt_d = nc.dram_tensor("xt", [P, NTILES * 3 * FT], bf16,
                          kind="ExternalInput")
    acc_d = nc.dram_tensor("acc", [P, NTILES * NACC], f32,
                           kind="ExternalOutput")

    with tile.TileContext(nc) as tc, ExitStack() as ctx:
        io = ctx.enter_context(tc.tile_pool(name="io", bufs=2))
        mid = ctx.enter_context(tc.tile_pool(name="mid", bufs=2))
        accp = ctx.enter_context(tc.tile_pool(name="accp", bufs=1))
        acc = accp.tile([P, NTILES * NACC], f32)

        def body(_rep):
            for ti in range(NTILES):
                cls, j = divmod(ti, TPC)
                # q on ACT for the pad-carrying tiles (j == TPC-1) so the
                # host pad replay follows one formula; q on DVE otherwise
                # to balance the two engines.
                q_on_act = (j == TPC - 1)

                x = io.tile([P, 3 * FT], bf16, tag="x")
                nc.sync.dma_start(
                    x[:], xt_d.ap()[:, ti * 3 * FT:(ti + 1) * 3 * FT])
                xv = (x[:, 0:FT], x[:, FT:2 * FT], x[:, 2 * FT:3 * FT])

                def ac(jj):
                    return acc[:, ti * NACC + jj: ti * NACC + jj + 1]

                # softmax denominator: exp on ACT, adds on gpsimd
                e = mid.tile([P, 3 * FT], f32, tag="e")
                nc.scalar.activation(e[:], x[:], AF.Exp)
                s01 = mid.tile([P, FT], f32, tag="s01")
                nc.gpsimd.tensor_tensor(s01[:], e[:, 0:FT], e[:, FT:2 * FT],
                                        A.add)
                s = mid.tile([P, FT], f32, tag="s")
                nc.gpsimd.tensor_tensor(s[:], s01[:], e[:, 2 * FT:3 * FT],
                                        A.add)
                lse = mid.tile([P, FT], bf16, tag="lse")
                nc.scalar.activation(lse[:], s[:], AF.Ln)

                # ce = lse - x_cls ; accumulate sum(ce)
                ce = mid.tile([P, FT], bf16, tag="ce")
                nc.vector.tensor_tensor(ce[:], lse[:], xv[cls], A.subtract)
                scr = mid.tile([P, FT], bf16, tag="scr")
                nc.vector.tensor_scalar(scr[:], ce[:], 0.0, None,
                                        op0=A.bypass, op1=A.add,
                                        accum_out=ac(0))

                # focal: pt = e^{-ce}; q = (1-pt)^2; fo = q*ce
                pt = mid.tile([P, FT], bf16, tag="pt")
                nc.scalar.activation(pt[:], ce[:], AF.Exp, scale=-1.0)
                q = mid.tile([P, FT], bf16, tag="q")
                if q_on_act:
                    nc.scalar.activation(q[:], pt[:], AF.Square, bias=1.0,
                                         scale=-1.0)
                else:
                    f1 = mid.tile([P, FT], bf16, tag="f1")
                    nc.vector.tensor_scalar(f1[:], pt[:], -1.0, 1.0,
                                            op0=A.mult, op1=A.add)
                    nc.vector.tensor_tensor(q[:], f1[:], f1[:], A.mult)
                fo = mid.tile([P, FT], bf16, tag="fo")
                nc.vector.tensor_tensor(fo[:], q[:], ce[:], A.mult)
                nc.vector.tensor_scalar(scr[:], fo[:], 0.0, None,
                                        op0=A.bypass, op1=A.add,
                                        accum_out=ac(1))

                # pred masks: c0 = max(x1,x2) > x0 ; is1 = c0 * (x1 >= x2)
                c12 = mid.tile([P, FT], bf16, tag="c12")
                nc.vector.tensor_tensor(c12[:], xv[1], xv[2], A.is_ge)
                m12 = mid.tile([P, FT], bf16, tag="m12")
                nc.vector.tensor_tensor(m12[:], xv[1], xv[2], A.max)
                c0 = mid.tile([P, FT], bf16, tag="c0")
                nc.vector.tensor_tensor(c0[:], m12[:], xv[0], A.is_gt)
                is1 = mid.tile([P, FT], bf16, tag="is1")
                nc.vector.tensor_tensor(is1[:], c0[:], c12[:], A.mult)
                nc.vector.tensor_scalar(scr[:], is1[:], 0.0, None,
                                        op0=A.bypass, op1=A.add,
                                        accum_out=ac(2))
                nc.vector.tensor_scalar(scr[:], c0[:], 0.0, None,
                                        op0=A.bypass, op1=A.add,
                                        accum_out=ac(3))

        if timing_loop and repeat > 1:
            with tc.For_i(0, repeat, 1):
                body(0)
        else:
            for r in range(repeat):
                body(r)

        nc.sync.dma_start(acc_d.ap()[:], acc[:])

    nc.compile()
    return nc


def _get_nc(repeat: int = 1, timing_loop: bool = False):
    key = (repeat, timing_loop)
    if key not in _nc_cache:
        _nc_cache[key] = _build(repeat, timing_loop)
    return _nc_cache[key]


def _prep_in_maps(outputs, targets):
    """Bucket each core's shard by class into padded column-major segments,
    then lay out DRAM as [P, NTILES, 3, FT] so each device tile is one
    contiguous DMA. Pad samples are x=(0,0,0) at the tail columns of each
    class segment. Returns (in_maps, counts[NCORES,3])."""
    xb = np.asarray(outputs).astype(BF16)
    tg = np.asarray(targets)
    in_maps = []
    counts = np.zeros((NCORES, 3), dtype=np.int64)
    for c in range(NCORES):
        lo, hi = c * BC, (c + 1) * BC
        xc = xb[lo:hi]
        tc_ = tg[lo:hi]
        xt = np.zeros((P, NTILES, 3, FT), dtype=BF16)
        for cls in range(3):
            sel = xc[tc_ == cls]                      # [n, 3]
            n = sel.shape[0]
            counts[c, cls] = n
            if n > P * CAPC:
                raise ValueError(f"class {cls} count {n} exceeds capacity")
            if n <= P * FT * (TPC - 1):
                raise ValueError(f"class {cls} count {n} too small for "
                                 "pad-tile assumption")
            buf = np.zeros((P * CAPC, 3), dtype=BF16)
            buf[:n] = sel
            # column-major fill: sample k -> (row k%P, col k//P) so pads
            # land in the last columns (the j==TPC-1 tile).
            seg = buf.reshape(CAPC, P, 3).transpose(1, 0, 2)  # [P, CAPC, 3]
            for j in range(TPC):
                ti = cls * TPC + j
                blk = seg[:, j * FT:(j + 1) * FT, :]          # [P, FT, 3]
                xt[:, ti] = blk.transpose(0, 2, 1)            # [P, 3, FT]
        in_maps.append({"xt": xt.reshape(P, NTILES * 3 * FT)})
    return in_maps, counts


def _pad_contrib():
    """Replay the device's arithmetic for the pad sample x=(0,0,0),
    mirroring dtype boundaries (bf16 tiles, f32 accumulation)."""
    bf = BF16
    lse = np.float32(np.log(np.float32(3.0))).astype(bf)   # Ln -> bf16 tile
    ce = (lse.astype(np.float32) - np.float32(0.0)).astype(bf)
    pt = np.exp(-ce.astype(np.float32)).astype(bf)
    q = ((np.float32(1.0) - pt.astype(np.float32)) ** 2).astype(bf)  # ACT q
    fo = (q.astype(np.float32) * ce.astype(np.float32)).astype(bf)
    return float(ce.astype(np.float64)), float(fo.astype(np.float64))


def _combine(accs, counts, class_weights, penalty_matrix):
    """accs: per-core [P, NTILES*NACC]; counts: [NCORES, 3] -> loss."""
    ce_c = np.zeros(3, dtype=np.float64)
    i1_c = np.zeros(3, dtype=np.float64)
    c0_c = np.zeros(3, dtype=np.float64)
    fo_sum = 0.0
    for a in accs:
        t = a.astype(np.float64).reshape(P, NTILES, NACC).sum(axis=0)
        for ti in range(NTILES):
            cls = ti // TPC
            ce_c[cls] += t[ti, 0]
            fo_sum += t[ti, 1]
            i1_c[cls] += t[ti, 2]
            c0_c[cls] += t[ti, 3]

    n_c = counts.sum(axis=0).astype(np.float64)           # true class counts
    npad_c = NCORES * P * CAPC - n_c                       # pads per class
    ce_pad, fo_pad = _pad_contrib()
    ce_c = ce_c - npad_c * ce_pad
    fo_sum = fo_sum - npad_c.sum() * fo_pad
    # pads have pred==0 -> contribute 0 to is1/c0 sums

    i2_c = c0_c - i1_c                                     # pred==2 per class
    N = float(B)
    w = class_weights.astype(np.float64)
    Pm = penalty_matrix.astype(np.float64)

    S_w = (w * n_c).sum()
    S_wce = (w * ce_c).sum()
    ce_loss = S_wce / S_w

    focal_loss = ALPHA * fo_sum / N

    # penalty: sum over (true class c, pred j) of P[c,j] * count(c,j)
    i0_c = n_c - i1_c - i2_c                               # pred==0 per class
    S_pen = (Pm[:, 0] * i0_c + Pm[:, 1] * i1_c + Pm[:, 2] * i2_c).sum()
    safety_penalty = S_pen / N

    n_crit = n_c[2]
    misses = n_c[2] - i2_c[2]
    critical = (misses / max(n_crit, 1.0)) * CRIT_PENALTY if n_crit > 0 else 0.0

    total = ce_loss + 0.3 * focal_loss + 0.4 * safety_penalty + 0.6 * critical
    return np.float32(total)


def kernel(outputs, targets, class_weights, penalty_matrix):
    nc = _get_nc(1)
    in_maps, counts = _prep_in_maps(outputs, targets)
    res = bass_utils.run_bass_kernel_spmd(nc, in_maps,
                                          core_ids=list(range(NCORES)))
    accs = [res.results[c]["acc"] for c in range(NCORES)]
    return _combine(accs, counts, np.asarray(class_weights),
                    np.asarray(penalty_matrix))


# revision 17
# speedup vs baseline: 1.2363x; 1.2363x over previous
"""Trainium2 Bass kernel for AdvancedClinicalSafetyLoss.

Strategy: pure data parallel over 8 NeuronCores, with host-side class
bucketing as the sharding layout. The loss is an order-independent sum
over samples, so inside each core's shard the host groups samples by
target class into three fixed-capacity column segments (padded with the
neutral sample x=(0,0,0) whose contribution is replayed exactly on the
host). Every device tile is then single-class at COMPILE TIME:

  - x_t gather disappears (ce = lse - x_cls directly),
  - all target masks disappear,
  - every masked sum (w_t*ce, per-(t,pred) penalty counts, critical
    hits) collapses into per-class plain sums accumulated for free by
    tensor_scalar accum_out.

Device math per tile of class c (logits in bf16, 2x/4x DVE modes):
  e = exp(x)  (ACT, one op over [128, 3F])    s = e0+e1+e2  (gpsimd)
  lse = ln(s) (bf16)                          ce = lse - x_c      [TT]
  pt = exp(-ce) (ACT)                         q = (1-pt)^2  (ACT or DVE)
  fo = q*ce [TT]                              c12 = x1>=x2 [TT]
  m12 = max(x1,x2) [TT]                       c0 = m12 > x0 [TT]
  is1 = c0*c12 [TT]   (pred==1; pred==2 = c0-is1 via host algebra)
Per-tile accums (f32): sum(ce), sum(fo), sum(is1), sum(c0).

Host (float64) reconstructs: weighted-CE ratio from per-class ce sums,
focal mean, penalty matrix bilinear over per-class is1/c0 sums, and the
critical-miss ratio; class counts come from the host's own bucketing.
"""

from contextlib import ExitStack

import numpy as np
import ml_dtypes

import concourse.bass as bass
import concourse.tile as tile
from concourse import bacc, mybir
from concourse import bass_utils

B = 8388608
NCORES = 8
P = 128
BC = B // NCORES            # samples per core
FT = 1408                   # columns per tile
TPC = 2                     # tiles per class segment
CAPC = FT * TPC             # columns per class segment (2816)
NTILES = 3 * TPC
NACC = 4                    # ce, fo, is1, c0 per tile

ALPHA = 0.25
CRIT_PENALTY = 50.0

BF16 = ml_dtypes.bfloat16

_nc_cache = {}


def _patch_act_tables():
    """Make exp/ln/square resolve to the one table set that holds all
    three (natural_log_exp_and_others) so the ACT engine does a single
    table load instead of thrashing between per-function sets.

    The set-id is positional (index into act_info.json), so the dict
    ORDER must be preserved; we instead strip these functions from every
    other set so the placement pass can only pick the combined one."""
    import concourse.bacc as bacc_mod
    import concourse.hw_specs as hw_specs
    if getattr(bacc_mod.get_activation_tables, "_combined_only", False):
        return
    orig = hw_specs.get_activation_tables
    AF = mybir.ActivationFunctionType
    moved = {AF.Exp, AF.Ln, AF.Square}
    pref = "natural_log_exp_and_others"

    def stripped(arch):
        t = orig(arch)
        if pref not in t or not moved <= t[pref]:
            return t
        return {k: (v if k == pref else v - moved) for k, v in t.items()}

    stripped._combined_only = True
    bacc_mod.get_activation_tables = stripped


def _build(repeat: int = 1, timing_loop: bool = False):
    """Build + compile the per-core Bass program (SPMD, same on all cores)."""
    _patch_act_tables()
    f32 = mybir.dt.float32
    bf16 = mybir.dt.bfloat16
    A = mybir.AluOpType
    AF = mybir.ActivationFunctionType

    nc = bacc.Bacc("TRN2", target_bir_lowering=False, debug=False,
                   num_devices=NCORES)
    # xt_d row layout per partition: [NTILES, 3 logit-streams, FT] so each
    # tile is one contiguous [P, 3*FT] DMA.
    xt_d = nc.dram_tensor("xt", [P, NTILES * 3 * FT], bf16,
                          kind="ExternalInput")
    acc_d = nc.dram_tensor("acc", [P, NTILES * NACC], f32,
                           kind="ExternalOutput")

    with tile.TileContext(nc) as tc, ExitStack() as ctx:
        io = ctx.enter_context(tc.tile_pool(name="io", bufs=2))
        mid = ctx.enter_context(tc.tile_pool(name="mid", bufs=2))
        accp = ctx.enter_context(tc.tile_pool(name="accp", bufs=1))
        acc = accp.tile([P, NTILES * NACC], f32)

        def body(_rep):
            for ti in range(NTILES):
                cls, j = divmod(ti, TPC)
                # q on ACT for the pad-carrying tiles (j == TPC-1) so the
                # host pad replay follows one formula; q on DVE otherwise
                # to balance the two engines.
                q_on_act = (j == TPC - 1)

                x = io.tile([P, 3 * FT], bf16, tag="x")
                nc.sync.dma_start(
                    x[:], xt_d.ap()[:, ti * 3 * FT:(ti + 1) * 3 * FT])
                xv = (x[:, 0:FT], x[:, FT:2 * FT], x[:, 2 * FT:3 * FT])

                def ac(jj):
                    return acc[:, ti * NACC + jj: ti * NACC + jj + 1]

                # pred masks first: independent of the exp/ln chain, so
                # DVE starts immediately while ACT runs exp
                scr2 = mid.tile([P, FT], bf16, tag="scr2")
                c12 = mid.tile([P, FT], bf16, tag="c12")
                nc.vector.tensor_tensor(c12[:], xv[1], xv[2], A.is_ge)
                m12 = mid.tile([P, FT], bf16, tag="m12")
                nc.vector.tensor_tensor(m12[:], xv[1], xv[2], A.max)
                c0 = mid.tile([P, FT], bf16, tag="c0")
                nc.vector.tensor_tensor(c0[:], m12[:], xv[0], A.is_gt)
                is1 = mid.tile([P, FT], bf16, tag="is1")
                nc.vector.tensor_tensor(is1[:], c0[:], c12[:], A.mult)
                nc.vector.tensor_scalar(scr2[:], is1[:], 0.0, None,
                                        op0=A.bypass, op1=A.add,
                                        accum_out=ac(2))
                nc.vector.tensor_scalar(scr2[:], c0[:], 0.0, None,
                                        op0=A.bypass, op1=A.add,
                                        accum_out=ac(3))

                # softmax denominator: exp on ACT, adds on gpsimd
                e = mid.tile([P, 3 * FT], f32, tag="e")
                nc.scalar.activation(e[:], x[:], AF.Exp)
                s01 = mid.tile([P, FT], f32, tag="s01")
                nc.gpsimd.tensor_tensor(s01[:], e[:, 0:FT], e[:, FT:2 * FT],
                                        A.add)
                s = mid.tile([P, FT], f32, tag="s")
                nc.gpsimd.tensor_tensor(s[:], s01[:], e[:, 2 * FT:3 * FT],
                                        A.add)
                lse = mid.tile([P, FT], bf16, tag="lse")
                nc.scalar.activation(lse[:], s[:], AF.Ln)

                # ce = lse - x_cls ; accumulate sum(ce)
                ce = mid.tile([P, FT], bf16, tag="ce")
                nc.vector.tensor_tensor(ce[:], lse[:], xv[cls], A.subtract)
                scr = mid.tile([P, FT], bf16, tag="scr")
                nc.vector.tensor_scalar(scr[:], ce[:], 0.0, None,
                                        op0=A.bypass, op1=A.add,
                                        accum_out=ac(0))

                # focal: pt = e^{-ce}; q = (1-pt)^2; fo = q*ce
                pt = mid.tile([P, FT], bf16, tag="pt")
                nc.scalar.activation(pt[:], ce[:], AF.Exp, scale=-1.0)
                q = mid.tile([P, FT], bf16, tag="q")
                if q_on_act:
                    nc.scalar.activation(q[:], pt[:], AF.Square, bias=1.0,
                                         scale=-1.0)
                else:
                    f1 = mid.tile([P, FT], bf16, tag="f1")
                    nc.vector.tensor_scalar(f1[:], pt[:], -1.0, 1.0,
                                            op0=A.mult, op1=A.add)
                    nc.vector.tensor_tensor(q[:], f1[:], f1[:], A.mult)
                fo = mid.tile([P, FT], bf16, tag="fo")
                nc.vector.tensor_tensor(fo[:], q[:], ce[:], A.mult)
                nc.vector.tensor_scalar(scr[:], fo[:], 0.0, None,
                                        op0=A.bypass, op1=A.add,
                                        accum_out=ac(1))


        if timing_loop and repeat > 1:
            with tc.For_i(0, repeat, 1):
                body(0)
        else:
            for r in range(repeat):
                body(r)

        nc.sync.dma_start(acc_d.ap()[:], acc[:])

    nc.compile()
    return nc


def _get_nc(repeat: int = 1, timing_loop: bool = False):
    key = (repeat, timing_loop)
    if key not in _nc_cache:
        _nc_cache[key] = _build(repeat, timing_loop)
    return _nc_cache[key]


def _prep_in_maps(outputs, targets):
    """Bucket each core's shard by class into padded column-major segments,
    then lay out DRAM as [P, NTILES, 3, FT] so each device tile is one
    contiguous DMA. Pad samples are x=(0,0,0) at the tail columns of each
    class segment. Returns (in_maps, counts[NCORES,3])."""
    xb = np.asarray(outputs).astype(BF16)
    tg = np.asarray(targets)
    in_maps = []
    counts = np.zeros((NCORES, 3), dtype=np.int64)
    for c in range(NCORES):
        lo, hi = c * BC, (c + 1) * BC
        xc = xb[lo:hi]
        tc_ = tg[lo:hi]
        xt = np.zeros((P, NTILES, 3, FT), dtype=BF16)
        for cls in range(3):
            sel = xc[tc_ == cls]                      # [n, 3]
            n = sel.shape[0]
            counts[c, cls] = n
            if n > P * CAPC:
                raise ValueError(f"class {cls} count {n} exceeds capacity")
            if n <= P * FT * (TPC - 1):
                raise ValueError(f"class {cls} count {n} too small for "
                                 "pad-tile assumption")
            buf = np.zeros((P * CAPC, 3), dtype=BF16)
            buf[:n] = sel
            # column-major fill: sample k -> (row k%P, col k//P) so pads
            # land in the last columns (the j==TPC-1 tile).
            seg = buf.reshape(CAPC, P, 3).transpose(1, 0, 2)  # [P, CAPC, 3]
            for j in range(TPC):
                ti = cls * TPC + j
                blk = seg[:, j * FT:(j + 1) * FT, :]          # [P, FT, 3]
                xt[:, ti] = blk.transpose(0, 2, 1)            # [P, 3, FT]
        in_maps.append({"xt": xt.reshape(P, NTILES * 3 * FT)})
    return in_maps, counts


def _pad_contrib():
    """Replay the device's arithmetic for the pad sample x=(0,0,0),
    mirroring dtype boundaries (bf16 tiles, f32 accumulation)."""
    bf = BF16
    lse = np.float32(np.log(np.float32(3.0))).astype(bf)   # Ln -> bf16 tile
    ce = (lse.astype(np.float32) - np.float32(0.0)).astype(bf)
    pt = np.exp(-ce.astype(np.float32)).astype(bf)
    q = ((np.float32(1.0) - pt.astype(np.float32)) ** 2).astype(bf)  # ACT q
    fo = (q.astype(np.float32) * ce.astype(np.float32)).astype(bf)
    return float(ce.astype(np.float64)), float(fo.astype(np.float64))


def _combine(accs, counts, class_weights, penalty_matrix):
    """accs: per-core [P, NTILES*NACC]; counts: [NCORES, 3] -> loss."""
    ce_c = np.zeros(3, dtype=np.float64)
    i1_c = np.zeros(3, dtype=np.float64)
    c0_c = np.zeros(3, dtype=np.float64)
    fo_sum = 0.0
    for a in accs:
        t = a.astype(np.float64).reshape(P, NTILES, NACC).sum(axis=0)
        for ti in range(NTILES):
            cls = ti // TPC
            ce_c[cls] += t[ti, 0]
            fo_sum += t[ti, 1]
            i1_c[cls] += t[ti, 2]
            c0_c[cls] += t[ti, 3]

    n_c = counts.sum(axis=0).astype(np.float64)           # true class counts
    npad_c = NCORES * P * CAPC - n_c                       # pads per class
    ce_pad, fo_pad = _pad_contrib()
    ce_c = ce_c - npad_c * ce_pad
    fo_sum = fo_sum - npad_c.sum() * fo_pad
    # pads have pred==0 -> contribute 0 to is1/c0 sums

    i2_c = c0_c - i1_c                                     # pred==2 per class
    N = float(B)
    w = class_weights.astype(np.float64)
    Pm = penalty_matrix.astype(np.float64)

    S_w = (w * n_c).sum()
    S_wce = (w * ce_c).sum()
    ce_loss = S_wce / S_w

    focal_loss = ALPHA * fo_sum / N

    # penalty: sum over (true class c, pred j) of P[c,j] * count(c,j)
    i0_c = n_c - i1_c - i2_c                               # pred==0 per class
    S_pen = (Pm[:, 0] * i0_c + Pm[:, 1] * i1_c + Pm[:, 2] * i2_c).sum()
    safety_penalty = S_pen / N

    n_crit = n_c[2]
    misses = n_c[2] - i2_c[2]
    critical = (misses / max(n_crit, 1.0)) * CRIT_PENALTY if n_crit > 0 else 0.0

    total = ce_loss + 0.3 * focal_loss + 0.4 * safety_penalty + 0.6 * critical
    return np.float32(total)


def kernel(outputs, targets, class_weights, penalty_matrix):
    nc = _get_nc(1)
    in_maps, counts = _prep_in_maps(outputs, targets)
    res = bass_utils.run_bass_kernel_spmd(nc, in_maps,
                                          core_ids=list(range(NCORES)))
    accs = [res.results[c]["acc"] for c in range(NCORES)]
    return _combine(accs, counts, np.asarray(class_weights),
                    np.asarray(penalty_matrix))
